# revision 1
# baseline (speedup 1.0000x reference)
"""Trainium2 Bass kernel: cross-modal channel attention.

Math (per batch b), with G the static [L, S] linear-interp matrix:
    q    = img_feat[b] reshaped [C, S]            (C=768, S=1024, L=77, D=512)
    tp   = text_feat[b] @ W_txt                   [L, C]
    t    = tp^T @ G                               [C, S]   (never materialized)
    logits^T = t @ q^T = tp^T @ (G @ q^T)         [Cj, Ci]  -- factored via L
    E^T  = exp(logits^T * S^-0.5)                 [Cj, Ci]
    Z_i  = sum_j E^T[j, i]   (ones-column matmuls)
    outA = E @ t = (tp @ E^T)^T @ G               [Ci, S]   -- factored via L
    out  = q + (gamma / Z_i) * outA               [C, S]

Sharding: data-parallel over batch across 8 cores (4 batches/core);
W_txt, G, gamma replicated.  Matmuls run in float32r (TF32 PE fast path,
1 cycle/row); fp32r operands are produced by rounding PSUM->SBUF copies /
activations.  The residual q stays exact fp32.
"""

import sys

sys.path.insert(0, "/opt/trn_rl_repo")

from contextlib import ExitStack

import numpy as np

import concourse.bacc as bacc
import concourse.mybir as mybir
import concourse.tile as tile
from concourse.bass_utils import run_bass_kernel_spmd
from concourse.masks import make_identity

B, C, HH, WW = 32, 768, 32, 32
S = HH * WW
L, D = 77, 512
N_CORES = 8
B_CORE = B // N_CORES
P = 128
CT, ST, DT = C // P, S // P, D // P
F32 = mybir.dt.float32
F32R = mybir.dt.float32r
SCALE = float(S) ** -0.5
EXP = mybir.ActivationFunctionType.Exp
MULT = mybir.AluOpType.mult
ADD = mybir.AluOpType.add


def _round_tf32(x):
    """Round fp32 -> tf32-representable (10-bit mantissa, round-to-nearest-even)."""
    u = np.ascontiguousarray(x, dtype=np.float32).view(np.uint32)
    r = (u + np.uint32(0x0FFF) + ((u >> np.uint32(13)) & np.uint32(1))) & np.uint32(
        0xFFFFE000
    )
    return r.view(np.float32)


def _interp_matrix():
    """G[l, s] such that (tp^T @ G)[c, s] == linear_interp(tp^T, S)[c, s]."""
    src = np.clip(
        (np.arange(S, dtype=np.float32) + np.float32(0.5)) * np.float32(L / S)
        - np.float32(0.5),
        np.float32(0.0),
        np.float32(L - 1),
    )
    i0 = np.floor(src).astype(np.int32)
    i1 = np.minimum(i0 + 1, L - 1)
    w = (src - i0.astype(np.float32)).astype(np.float32)
    g = np.zeros((L, S), dtype=np.float32)
    g[i0, np.arange(S)] += np.float32(1.0) - w
    g[i1, np.arange(S)] += w
    return g


def _build():
    nc = bacc.Bacc("TRN2", target_bir_lowering=False, debug=False)
    img = nc.dram_tensor("img", [B_CORE, C, S], F32, kind="ExternalInput").ap()
    txt = nc.dram_tensor("txt", [B_CORE, L, D], F32, kind="ExternalInput").ap()
    wt = nc.dram_tensor("wt", [D, C], F32R, kind="ExternalInput").ap()
    g = nc.dram_tensor("g", [L, S], F32R, kind="ExternalInput").ap()
    gt = nc.dram_tensor("gt", [S, L], F32R, kind="ExternalInput").ap()
    gamma = nc.dram_tensor("gamma128", [P, 1], F32, kind="ExternalInput").ap()
    out = nc.dram_tensor("out", [B_CORE, C, S], F32, kind="ExternalOutput").ap()

    with ExitStack() as ctx:
        tc = ctx.enter_context(tile.TileContext(nc))
        consts = ctx.enter_context(tc.tile_pool(name="consts", bufs=1))
        q_pool = ctx.enter_context(tc.tile_pool(name="q", bufs=2))
        txt_pool = ctx.enter_context(tc.tile_pool(name="txtp", bufs=2))
        small = ctx.enter_context(tc.tile_pool(name="small", bufs=2))
        qtb_pool = ctx.enter_context(tc.tile_pool(name="qtb", bufs=3))
        et_pool = ctx.enter_context(tc.tile_pool(name="et", bufs=2))
        outp = ctx.enter_context(tc.tile_pool(name="outp", bufs=2))
        zp = ctx.enter_context(tc.tile_pool(name="zp", bufs=3))
        # PSUM: small pool 2x1 bank + big pool 2x3 banks = 8 banks total.
        ps_small = ctx.enter_context(tc.tile_pool(name="ps_s", bufs=2, space="PSUM"))
        ps_big = ctx.enter_context(tc.tile_pool(name="ps_b", bufs=2, space="PSUM"))

        w_sb = consts.tile([P, DT, C], F32R)
        nc.sync.dma_start(w_sb[:], wt.rearrange("(k p) c -> p k c", p=P))
        g_sb = consts.tile([P, S], F32R)
        nc.sync.dma_start(g_sb[0:L, :], g)
        gt_sb = consts.tile([P, ST, L], F32R)
        nc.sync.dma_start(gt_sb[:], gt.rearrange("(st p) l -> p st l", p=P))
        gamma_sb = consts.tile([P, 1], F32)
        nc.sync.dma_start(gamma_sb[:], gamma)
        ident = consts.tile([P, P], F32)
        make_identity(nc, ident[:])
        # f32r memset/affine_select fail codegen -> produce via rounding copies
        ident_r = consts.tile([P, P], F32R)
        nc.vector.tensor_copy(ident_r[:], ident[:])
        ones_f = consts.tile([P, 2], F32)
        nc.gpsimd.memset(ones_f[:], 1.0)
        ones_sb = consts.tile([P, 2], F32R)
        nc.vector.tensor_copy(ones_sb[:], ones_f[:])

        for b in range(B_CORE):
            q_sb = q_pool.tile([P, CT, S], F32, tag="q")
            nc.sync.dma_start(q_sb[:], img[b].rearrange("(ct p) s -> p ct s", p=P))
            txt_sb = txt_pool.tile([P, D], F32, tag="txt")
            nc.sync.dma_start(txt_sb[0:L, :], txt[b])

            # text^T [D, L] via fp32 PE transposes; rounding copy -> f32r
            ps_tt = ps_small.tile([P, DT, P], F32, tag="ps")
            for k in range(DT):
                nc.tensor.transpose(
                    ps_tt[:, k, 0:L],
                    txt_sb[0:L, k * P : (k + 1) * P],
                    ident[0:L, 0:L],
                )
            txtT_sb = small.tile([P, DT, P], F32R, tag="txtT")
            nc.vector.tensor_copy(txtT_sb[:, :, 0:L], ps_tt[:, :, 0:L])

            # tp = text @ W_txt  [L, C]
            tp_sb = small.tile([P, C], F32R, tag="tp")
            ps_a = ps_small.tile([P, 512], F32, tag="ps")
            for k in range(DT):
                nc.tensor.matmul(
                    ps_a[0:L, :],
                    txtT_sb[:, k, 0:L],
                    w_sb[:, k, 0:512],
                    start=(k == 0),
                    stop=(k == DT - 1),
                )
            nc.scalar.copy(tp_sb[0:L, 0:512], ps_a[0:L, :])
            ps_b2 = ps_small.tile([P, 512], F32, tag="ps")
            for k in range(DT):
                nc.tensor.matmul(
                    ps_b2[0:L, 0:256],
                    txtT_sb[:, k, 0:L],
                    w_sb[:, k, 512:768],
                    start=(k == 0),
                    stop=(k == DT - 1),
                )
            nc.scalar.copy(tp_sb[0:L, 512:768], ps_b2[0:L, 0:256])

            # tp^T [C, L] via f32r PE transposes of tp
            # tp^T via regular matmul against identity (fp32r dst must be even -> N=78)
            ps_tp = ps_small.tile([P, CT, 80], F32, tag="ps")
            for jt in range(CT):
                nc.tensor.matmul(
                    ps_tp[:, jt, 0 : L + 1],
                    tp_sb[0:L, jt * P : (jt + 1) * P],
                    ident_r[0:L, 0 : L + 1],
                    start=True,
                    stop=True,
                )
            tpT_sb = small.tile([P, CT, 80], F32R, tag="tpT")
            nc.vector.tensor_copy(tpT_sb[:, :, 0:L], ps_tp[:, :, 0:L])

            # q^T blocks (streamed) + GQT = G @ q^T  [L, C] accumulated over S
            ps_gqt = ps_big.tile([P, 1025], F32, tag="psb")
            for st in range(ST):
                ps1 = ps_small.tile([P, 512], F32, tag="ps")
                for ct in range(4):
                    nc.tensor.transpose(
                        ps1[:, ct * P : (ct + 1) * P],
                        q_sb[:, ct, st * P : (st + 1) * P],
                        ident[:],
                    )
                ps2 = ps_small.tile([P, 512], F32, tag="ps")
                for ct in range(4, 6):
                    nc.tensor.transpose(
                        ps2[:, (ct - 4) * P : (ct - 3) * P],
                        q_sb[:, ct, st * P : (st + 1) * P],
                        ident[:],
                    )
                qtb = qtb_pool.tile([P, C], F32R, tag="qtb")
                if st % 2 == 0:
                    nc.vector.tensor_copy(qtb[:, 0:512], ps1[:, :])
                    nc.vector.tensor_copy(qtb[:, 512:768], ps2[:, 0:256])
                else:
                    nc.scalar.copy(qtb[:, 0:512], ps1[:, :])
                    nc.scalar.copy(qtb[:, 512:768], ps2[:, 0:256])
                nc.tensor.matmul(
                    ps_gqt[0:L, 0:512],
                    gt_sb[:, st, :],
                    qtb[:, 0:512],
                    start=(st == 0),
                    stop=(st == ST - 1),
                )
                nc.tensor.matmul(
                    ps_gqt[0:L, 512:768],
                    gt_sb[:, st, :],
                    qtb[:, 512:768],
                    start=(st == 0),
                    stop=(st == ST - 1),
                )
            gqt_sb = small.tile([P, C], F32R, tag="gqt")
            nc.scalar.copy(gqt_sb[0:L, :], ps_gqt[0:L, 0:C])

            # logits^T = tp^T @ GQT per j-tile, fused exp -> E^T (f32r)
            et_sb = et_pool.tile([P, CT, C], F32R, tag="et")
            for jt in range(CT):
                psl = ps_big.tile([P, 1025], F32, tag="psb")
                lhsT = tp_sb[0:L, jt * P : (jt + 1) * P]
                nc.tensor.matmul(
                    psl[:, 0:512], lhsT, gqt_sb[0:L, 0:512], start=True, stop=True
                )
                nc.tensor.matmul(
                    psl[:, 512:768], lhsT, gqt_sb[0:L, 512:768], start=True, stop=True
                )
                nc.scalar.activation(et_sb[:, jt, :], psl[:, 0:C], EXP, scale=SCALE)

            # ZT = tp @ E^T  [L, C] accumulated over j-tiles
            ps_z1 = ps_small.tile([P, 512], F32, tag="ps")
            ps_z2 = ps_small.tile([P, 512], F32, tag="ps")
            for jt in range(CT):
                nc.tensor.matmul(
                    ps_z1[0:L, :],
                    tpT_sb[:, jt, 0:L],
                    et_sb[:, jt, 0:512],
                    start=(jt == 0),
                    stop=(jt == CT - 1),
                )
                nc.tensor.matmul(
                    ps_z2[0:L, 0:256],
                    tpT_sb[:, jt, 0:L],
                    et_sb[:, jt, 512:768],
                    start=(jt == 0),
                    stop=(jt == CT - 1),
                )
            zt_sb = small.tile([P, C], F32R, tag="zt")
            nc.scalar.copy(zt_sb[0:L, 0:512], ps_z1[0:L, :])
            nc.scalar.copy(zt_sb[0:L, 512:768], ps_z2[0:L, 0:256])

            # outA = ZT^T @ G (+ Z_i from ones column), epilogue, store
            for it in range(CT):
                psa = ps_big.tile([P, 1026], F32, tag="psb")
                lhsT = zt_sb[0:L, it * P : (it + 1) * P]
                nc.tensor.matmul(
                    psa[:, 0:512], lhsT, g_sb[0:L, 0:512], start=True, stop=True
                )
                nc.tensor.matmul(
                    psa[:, 512:1024], lhsT, g_sb[0:L, 512:1024], start=True, stop=True
                )
                for jt in range(CT):
                    nc.tensor.matmul(
                        psa[:, 1024:1026],
                        et_sb[:, jt, it * P : (it + 1) * P],
                        ones_sb[:],
                        start=(jt == 0),
                        stop=(jt == CT - 1),
                    )
                rz = zp.tile([P, 1], F32, tag="rz")
                nc.vector.reciprocal(rz[:], psa[:, 1024:1025])
                gz = zp.tile([P, 1], F32, tag="gz")
                nc.vector.tensor_scalar_mul(gz[:], rz[:], gamma_sb[:])
                o_sb = outp.tile([P, S], F32, tag="o")
                nc.vector.scalar_tensor_tensor(
                    o_sb[:], psa[:, 0:1024], gz[:], q_sb[:, it, :], op0=MULT, op1=ADD
                )
                nc.sync.dma_start(
                    out[b].rearrange("(ct p) s -> ct p s", p=P)[it], o_sb[:]
                )

    nc.compile()
    return nc


_NC = None


def _get_nc():
    global _NC
    if _NC is None:
        _NC = _build()
    return _NC


def _in_maps(img_feat, text_feat, W_txt, gamma):
    img = np.ascontiguousarray(img_feat.reshape(B, C, S), dtype=np.float32)
    txt = np.ascontiguousarray(text_feat, dtype=np.float32)
    wt = _round_tf32(np.ascontiguousarray(W_txt, dtype=np.float32))
    g = _round_tf32(_interp_matrix())
    gt = np.ascontiguousarray(g.T)
    gamma128 = np.full((P, 1), np.float32(gamma.reshape(-1)[0]), dtype=np.float32)
    maps = []
    for m in range(N_CORES):
        sl = slice(m * B_CORE, (m + 1) * B_CORE)
        maps.append(
            {
                "img": np.ascontiguousarray(img[sl]),
                "txt": np.ascontiguousarray(txt[sl]),
                "wt": wt,
                "g": g,
                "gt": gt,
                "gamma128": gamma128,
            }
        )
    return maps


def _run(in_maps, **kwargs):
    nc = _get_nc()
    return run_bass_kernel_spmd(nc, in_maps, core_ids=list(range(N_CORES)), **kwargs)


def kernel(img_feat, text_feat, W_txt, gamma):
    res = _run(_in_maps(img_feat, text_feat, W_txt, gamma))
    full = np.concatenate([res.results[m]["out"] for m in range(N_CORES)], axis=0)
    return full.reshape(B, C, HH, WW).astype(np.float32)



# revision 12
# speedup vs baseline: 1.4545x; 1.4545x over previous
"""Trainium2 Bass kernel: cross-modal channel attention (transposed-space bf16).

Math (per batch b), with G the static [L, S] linear-interp matrix:
    qT   = img_feat[b]^T                          [S, C]  (xbar DMA-transpose load)
    tpT  = W^T-matmul: tpT[c,l] = sum_d W[d,c] txt[l,d]   [C, L]
    tp   = tpT^T (PE transpose)                   [L, C]
    GQT  = G @ qT                                 [L, C]
    E^T  = exp(tp^T @ GQT * S^-0.5)               [Cj, Ci]
    EP   = E @ tp^T (+ Z from ones column)        [Ci, L]
    EPs  = EP / Z ; EPsT = EPs^T                  [L, Ci]
    outT = qT + (gamma*G)^T @ EPsT                [S, C]  (host transposes back)

Sharding: data-parallel over batch across 8 cores (4 batches/core);
weights/interp matrices replicated.  All matmul operands bf16 (fp32 PSUM
accumulation); I/O in bf16 to halve HBM traffic; final transpose of the
output back to [C, S] happens on host (pure layout, no math).
"""

import sys

sys.path.insert(0, "/opt/trn_rl_repo")

from contextlib import ExitStack

import ml_dtypes
import numpy as np

import concourse.bacc as bacc
import concourse.mybir as mybir
import concourse.tile as tile
from concourse.bass_utils import run_bass_kernel_spmd
from concourse.masks import make_identity

B, C, HH, WW = 32, 768, 32, 32
S = HH * WW
L, D = 77, 512
LP = 80  # L padded (zero rows) for xbar/tile alignment
N_CORES = 8
B_CORE = B // N_CORES
P = 128
CT, ST, DT = C // P, S // P, D // P
F32 = mybir.dt.float32
BF16 = mybir.dt.bfloat16
SCALE = float(S) ** -0.5
EXP = mybir.ActivationFunctionType.Exp
ADD = mybir.AluOpType.add
BF = ml_dtypes.bfloat16


def _interp_matrix():
    """G[l, s] such that (tp^T @ G)[c, s] == linear_interp(tp^T, S)[c, s]."""
    src = np.clip(
        (np.arange(S, dtype=np.float32) + np.float32(0.5)) * np.float32(L / S)
        - np.float32(0.5),
        np.float32(0.0),
        np.float32(L - 1),
    )
    i0 = np.floor(src).astype(np.int32)
    i1 = np.minimum(i0 + 1, L - 1)
    w = (src - i0.astype(np.float32)).astype(np.float32)
    g = np.zeros((L, S), dtype=np.float32)
    g[i0, np.arange(S)] += np.float32(1.0) - w
    g[i1, np.arange(S)] += w
    return g


def _build():
    nc = bacc.Bacc("TRN2", target_bir_lowering=False, debug=False)
    img = nc.dram_tensor("img", [B_CORE, C, S], BF16, kind="ExternalInput").ap()
    txt = nc.dram_tensor("txt", [LP, B_CORE, D], BF16, kind="ExternalInput").ap()
    wt = nc.dram_tensor("wt", [P, DT, C], BF16, kind="ExternalInput").ap()
    gg = nc.dram_tensor("gg", [LP, S], BF16, kind="ExternalInput").ap()  # gamma*G
    gt = nc.dram_tensor("gt", [P, ST, LP], BF16, kind="ExternalInput").ap()  # G^T
    out = nc.dram_tensor("out", [B_CORE, S, C], BF16, kind="ExternalOutput").ap()

    with ExitStack() as ctx:
        tc = ctx.enter_context(tile.TileContext(nc))
        consts = ctx.enter_context(tc.tile_pool(name="consts", bufs=1))
        qt_pool = ctx.enter_context(tc.tile_pool(name="qt", bufs=2))
        txtT_pool = ctx.enter_context(tc.tile_pool(name="txtT", bufs=2))
        small = ctx.enter_context(tc.tile_pool(name="small", bufs=2))
        et_pool = ctx.enter_context(tc.tile_pool(name="et", bufs=2))
        outp = ctx.enter_context(tc.tile_pool(name="outp", bufs=2))
        zp = ctx.enter_context(tc.tile_pool(name="zp", bufs=3))
        # PSUM: 8 banks total = big 3x2 + small 2x1.
        ps_big = ctx.enter_context(tc.tile_pool(name="ps_b", bufs=3, space="PSUM"))
        ps_small = ctx.enter_context(tc.tile_pool(name="ps_s", bufs=2, space="PSUM"))

        w_sb = consts.tile([P, DT, C], BF16)
        nc.sync.dma_start(w_sb[:], wt)
        gg_sb = consts.tile([P, S], BF16)
        nc.sync.dma_start(gg_sb[0:LP, :], gg)
        gt_sb = consts.tile([P, ST, LP], BF16)
        nc.sync.dma_start(gt_sb[:], gt)
        txt_sb = consts.tile([P, B_CORE, D], BF16)
        nc.sync.dma_start(txt_sb[0:LP, :, :], txt)
        ident_f = consts.tile([P, P], F32)
        make_identity(nc, ident_f[:])
        ident = consts.tile([P, P], BF16)
        nc.vector.tensor_copy(ident[:], ident_f[:])

        for b in range(B_CORE):
            # qT tiles via xbar DMA transpose (compact 2D outputs required)
            qts = []
            for st in range(ST):
                q = qt_pool.tile([P, C], BF16, tag=f"qt{st}")
                nc.sync.dma_start_transpose(q[:], img[b][:, st * P : (st + 1) * P])
                qts.append(q)

            # GQT = G @ qT  [LP, C] accumulated over s-tiles (fp32 psum)
            ps_gqt = ps_big.tile([P, 768], F32, tag="psb")
            for st in range(ST):
                nc.tensor.matmul(
                    ps_gqt[0:LP, 0:512],
                    gt_sb[:, st, :],
                    qts[st][:, 0:512],
                    start=(st == 0),
                    stop=(st == ST - 1),
                )
                nc.tensor.matmul(
                    ps_gqt[0:LP, 512:768],
                    gt_sb[:, st, :],
                    qts[st][:, 512:768],
                    start=(st == 0),
                    stop=(st == ST - 1),
                )
            gqt_sb = small.tile([P, C], BF16, tag="gqt")
            nc.scalar.copy(gqt_sb[0:LP, :], ps_gqt[0:LP, 0:C])

            # txt^T tiles [D, LP] via PE transposes (bf16 psum)
            ps_tt = ps_small.tile([P, DT, LP], BF16, tag="pss")
            for k in range(DT):
                nc.tensor.transpose(
                    ps_tt[:, k, :],
                    txt_sb[0:LP, b, k * P : (k + 1) * P],
                    ident[0:LP, 0:LP],
                )
            txtT_sb = txtT_pool.tile([P, DT, LP], BF16, tag="txtT")
            nc.scalar.copy(txtT_sb[:], ps_tt[:])

            # tpT[c, l] = sum_d W[d, c] txt[l, d]   [C, LP] (fp32 psum)
            ps_tpT = ps_small.tile([P, CT, LP], F32, tag="pss")
            for k in range(DT):
                for ct in range(CT):
                    nc.tensor.matmul(
                        ps_tpT[:, ct, :],
                        w_sb[:, k, ct * P : (ct + 1) * P],
                        txtT_sb[:, k, :],
                        start=(k == 0),
                        stop=(k == DT - 1),
                    )
            # tpT_sb has LP+1 columns; col LP holds ones (Z accumulator column)
            tpT_sb = small.tile([P, CT, LP + 1], BF16, tag="tpT")
            nc.scalar.copy(tpT_sb[:, :, 0:LP], ps_tpT[:])
            nc.gpsimd.memset(
                tpT_sb[:, :, LP : LP + 1].rearrange("p a b -> p (a b)"), 1.0
            )

            # tp = tpT^T  [LP, C] via PE transposes (bf16 psum)
            ps_tp = ps_big.tile([P, 768], BF16, tag="psb")
            for jt in range(CT):
                nc.tensor.transpose(
                    ps_tp[0:LP, jt * P : (jt + 1) * P],
                    tpT_sb[:, jt, 0:LP],
                    ident[:],
                )
            tp_sb = small.tile([P, C], BF16, tag="tp")
            nc.scalar.copy(tp_sb[0:LP, :], ps_tp[0:LP, :])

            # logits^T per j-tile + fused exp -> E^T (bf16)
            et_sb = et_pool.tile([P, CT, C], BF16, tag="et")
            for jt in range(CT):
                psl = ps_big.tile([P, 768], F32, tag="psb")
                lhsT = tp_sb[0:LP, jt * P : (jt + 1) * P]
                nc.tensor.matmul(
                    psl[:, 0:512], lhsT, gqt_sb[0:LP, 0:512], start=True, stop=True
                )
                nc.tensor.matmul(
                    psl[:, 512:768], lhsT, gqt_sb[0:LP, 512:768], start=True, stop=True
                )
                nc.scalar.activation(et_sb[:, jt, :], psl[:, 0:C], EXP, scale=SCALE)

            # EP = E @ [tp^T | 1]  [Ci, LP+1]; col LP = Z_i  (fp32 psum)
            ps_ep = ps_small.tile([P, CT, LP + 1], F32, tag="pss")
            for jt in range(CT):
                for it in range(CT):
                    nc.tensor.matmul(
                        ps_ep[:, it, :],
                        et_sb[:, jt, it * P : (it + 1) * P],
                        tpT_sb[:, jt, :],
                        start=(jt == 0),
                        stop=(jt == CT - 1),
                    )
            # EPs = EP / Z  (bf16)
            eps_sb = small.tile([P, CT, LP], BF16, tag="eps")
            rz = zp.tile([P, CT], F32, tag="rz")
            nc.vector.reciprocal(
                rz[:], ps_ep[:, :, LP : LP + 1].rearrange("p a b -> p (a b)")
            )
            for it in range(CT):
                nc.vector.tensor_scalar_mul(
                    eps_sb[:, it, :], ps_ep[:, it, 0:LP], rz[:, it : it + 1]
                )

            # EPsT = EPs^T  [LP, C] via PE transposes (bf16 psum)
            ps_epsT = ps_big.tile([P, 768], BF16, tag="psb")
            for it in range(CT):
                nc.tensor.transpose(
                    ps_epsT[0:LP, it * P : (it + 1) * P],
                    eps_sb[:, it, :],
                    ident[:],
                )
            epsT_sb = small.tile([P, C], BF16, tag="epsT")
            nc.scalar.copy(epsT_sb[0:LP, :], ps_epsT[0:LP, :])

            # outT = qT + (gamma*G)^T @ EPsT  per s-tile; residual on DVE/Act/Pool
            o_sb = outp.tile([P, ST, C], BF16, tag="o")
            for st in range(ST):
                pso = ps_big.tile([P, 768], F32, tag="psb")
                lhsT = gg_sb[0:LP, st * P : (st + 1) * P]
                nc.tensor.matmul(
                    pso[:, 0:512], lhsT, epsT_sb[0:LP, 0:512], start=True, stop=True
                )
                nc.tensor.matmul(
                    pso[:, 512:768], lhsT, epsT_sb[0:LP, 512:768], start=True, stop=True
                )
                nc.vector.tensor_tensor(o_sb[:, st, :], pso[:, 0:C], qts[st][:], ADD)
            nc.sync.dma_start(out[b].rearrange("(st p) c -> p st c", p=P), o_sb[:])

    nc.compile()
    return nc


_NC = None


def _get_nc():
    global _NC
    if _NC is None:
        _NC = _build()
    return _NC


def _in_maps(img_feat, text_feat, W_txt, gamma):
    img = np.ascontiguousarray(
        img_feat.reshape(B, C, S), dtype=np.float32
    ).astype(BF)
    txt_p = np.zeros((LP, B, D), dtype=BF)
    txt_p[0:L] = np.asarray(text_feat, dtype=np.float32).transpose(1, 0, 2).astype(BF)
    wt = np.ascontiguousarray(
        np.asarray(W_txt, dtype=np.float32).reshape(DT, P, C).transpose(1, 0, 2)
    ).astype(BF)
    g = _interp_matrix()
    gam = np.float32(np.asarray(gamma).reshape(-1)[0])
    gg = np.zeros((LP, S), dtype=BF)
    gg[0:L] = (gam * g).astype(BF)
    gt = np.zeros((P, ST, LP), dtype=BF)
    gt[:, :, 0:L] = (
        g.T.reshape(ST, P, L).transpose(1, 0, 2).astype(BF)
    )
    maps = []
    for m in range(N_CORES):
        sl = slice(m * B_CORE, (m + 1) * B_CORE)
        maps.append(
            {
                "img": np.ascontiguousarray(img[sl]),
                "txt": np.ascontiguousarray(txt_p[:, sl]),
                "wt": wt,
                "gg": gg,
                "gt": gt,
            }
        )
    return maps


def _run(in_maps, **kwargs):
    nc = _get_nc()
    return run_bass_kernel_spmd(nc, in_maps, core_ids=list(range(N_CORES)), **kwargs)


def kernel(img_feat, text_feat, W_txt, gamma):
    res = _run(_in_maps(img_feat, text_feat, W_txt, gamma))
    full = np.concatenate(
        [np.asarray(res.results[m]["out"]) for m in range(N_CORES)], axis=0
    )  # [B, S, C] bf16
    full = full.astype(np.float32).transpose(0, 2, 1)
    return np.ascontiguousarray(full.reshape(B, C, HH, WW), dtype=np.float32)


# revision 14
# speedup vs baseline: 1.6179x; 1.1123x over previous
"""Trainium2 Bass kernel: cross-modal channel attention (transposed-space bf16).

Math (per batch b), with G the static [L, S] linear-interp matrix:
    qT   = img_feat[b]^T                          [S, C]  (xbar DMA-transpose load)
    tpT  = W^T-matmul: tpT[c,l] = sum_d W[d,c] txt[l,d]   [C, L]
    tp   = tpT^T (PE transpose)                   [L, C]
    GQT  = G @ qT                                 [L, C]
    E^T  = exp(tp^T @ GQT * S^-0.5)               [Cj, Ci]
    EP   = E @ [tp^T | 1]  (Z from ones column)   [Ci, L+1]
    EPs  = EP / Z ; EPsT = EPs^T                  [L, Ci]
    outT = qT + (gamma*G)^T @ EPsT                [S, C]  (host transposes back)

Sharding: data-parallel over batch across 8 cores (4 batches/core);
weights/interp matrices replicated.  All matmul operands bf16 (fp32 PSUM
accumulation); I/O in bf16 to halve HBM traffic; final transpose of the
output back to [C, S] happens on host (pure layout, no math).
"""

import sys

sys.path.insert(0, "/opt/trn_rl_repo")

from contextlib import ExitStack

import ml_dtypes
import numpy as np

import concourse.bacc as bacc
import concourse.mybir as mybir
import concourse.tile as tile
from concourse.bass_utils import run_bass_kernel_spmd
from concourse.masks import make_identity

B, C, HH, WW = 32, 768, 32, 32
S = HH * WW
L, D = 77, 512
LP = 80  # L padded (zero rows) for xbar/tile alignment
N_CORES = 8
B_CORE = B // N_CORES
P = 128
CT, ST, DT = C // P, S // P, D // P
F32 = mybir.dt.float32
BF16 = mybir.dt.bfloat16
SCALE = float(S) ** -0.5
EXP = mybir.ActivationFunctionType.Exp
ADD = mybir.AluOpType.add
BF = ml_dtypes.bfloat16
HC = C // 2  # half of C, for 1-bank psum tiles


def _interp_matrix():
    """G[l, s] such that (tp^T @ G)[c, s] == linear_interp(tp^T, S)[c, s]."""
    src = np.clip(
        (np.arange(S, dtype=np.float32) + np.float32(0.5)) * np.float32(L / S)
        - np.float32(0.5),
        np.float32(0.0),
        np.float32(L - 1),
    )
    i0 = np.floor(src).astype(np.int32)
    i1 = np.minimum(i0 + 1, L - 1)
    w = (src - i0.astype(np.float32)).astype(np.float32)
    g = np.zeros((L, S), dtype=np.float32)
    g[i0, np.arange(S)] += np.float32(1.0) - w
    g[i1, np.arange(S)] += w
    return g


def _build():
    nc = bacc.Bacc("TRN2", target_bir_lowering=False, debug=False)
    img = nc.dram_tensor("img", [B_CORE, C, S], BF16, kind="ExternalInput").ap()
    # text^T, host-prearranged: txtt[p, k, b, l] = text[b, l, k*128+p]
    txtt = nc.dram_tensor("txtt", [P, DT, B_CORE, LP], BF16, kind="ExternalInput").ap()
    wt = nc.dram_tensor("wt", [P, DT, C], BF16, kind="ExternalInput").ap()
    gg = nc.dram_tensor("gg", [LP, S], BF16, kind="ExternalInput").ap()  # gamma*G
    gt = nc.dram_tensor("gt", [P, ST, LP], BF16, kind="ExternalInput").ap()  # G^T
    out = nc.dram_tensor("out", [B_CORE, S, C], BF16, kind="ExternalOutput").ap()

    with ExitStack() as ctx:
        tc = ctx.enter_context(tile.TileContext(nc))
        consts = ctx.enter_context(tc.tile_pool(name="consts", bufs=1))
        qt_pool = ctx.enter_context(tc.tile_pool(name="qt", bufs=2))
        small = ctx.enter_context(tc.tile_pool(name="small", bufs=2))
        et_pool = ctx.enter_context(tc.tile_pool(name="et", bufs=2))
        outp = ctx.enter_context(tc.tile_pool(name="outp", bufs=2))
        zp = ctx.enter_context(tc.tile_pool(name="zp", bufs=3))
        # PSUM budget (8 banks): ph1 1 + ph3 1 + gqtA 1 + gqtB 1 + psl 2x1 + pso 2x1
        ps = ctx.enter_context(tc.tile_pool(name="ps", bufs=1, space="PSUM"))

        w_sb = consts.tile([P, DT, C], BF16)
        nc.sync.dma_start(w_sb[:], wt)
        gg_sb = consts.tile([P, S], BF16)
        nc.sync.dma_start(gg_sb[0:LP, :], gg)
        gt_sb = consts.tile([P, ST, LP], BF16)
        nc.sync.dma_start(gt_sb[:], gt)
        txtt_sb = consts.tile([P, DT, B_CORE, LP], BF16)
        nc.sync.dma_start(txtt_sb[:], txtt)
        ident_f = consts.tile([P, P], F32)
        make_identity(nc, ident_f[:])
        ident = consts.tile([P, P], BF16)
        nc.vector.tensor_copy(ident[:], ident_f[:])

        for b in range(B_CORE):
            # qT tiles via xbar DMA transpose (compact 2D outputs required)
            qts = []
            for st in range(ST):
                q = qt_pool.tile([P, C], BF16, tag=f"qt{st}")
                nc.sync.dma_start_transpose(q[:], img[b][:, st * P : (st + 1) * P])
                qts.append(q)

            # GQT = G @ qT  [LP, C] accumulated over s-tiles (fp32 psum)
            ps_gqtA = ps.tile([P, HC], F32, tag="gqtA")
            ps_gqtB = ps.tile([P, HC], F32, tag="gqtB")
            for st in range(ST):
                nc.tensor.matmul(
                    ps_gqtA[0:LP, :],
                    gt_sb[:, st, :],
                    qts[st][:, 0:HC],
                    start=(st == 0),
                    stop=(st == ST - 1),
                )
                nc.tensor.matmul(
                    ps_gqtB[0:LP, :],
                    gt_sb[:, st, :],
                    qts[st][:, HC:C],
                    start=(st == 0),
                    stop=(st == ST - 1),
                )
            gqt_sb = small.tile([P, C], BF16, tag="gqt")
            nc.scalar.copy(gqt_sb[0:LP, 0:HC], ps_gqtA[0:LP, :])
            nc.scalar.copy(gqt_sb[0:LP, HC:C], ps_gqtB[0:LP, :])

            # tpT[c, l] = sum_d W[d, c] txt[l, d]   [C, LP] (fp32 psum)
            ps_tpT = ps.tile([P, CT, LP + 1], F32, tag="phF")
            for k in range(DT):
                for ct in range(CT):
                    nc.tensor.matmul(
                        ps_tpT[:, ct, 0:LP],
                        w_sb[:, k, ct * P : (ct + 1) * P],
                        txtt_sb[:, k, b, :],
                        start=(k == 0),
                        stop=(k == DT - 1),
                    )
            # tpT_sb has LP+1 columns; col LP holds ones (Z accumulator column)
            tpT_sb = small.tile([P, CT, LP + 1], BF16, tag="tpT")
            nc.scalar.copy(tpT_sb[:, :, 0:LP], ps_tpT[:, :, 0:LP])
            nc.gpsimd.memset(
                tpT_sb[:, :, LP : LP + 1].rearrange("p a b -> p (a b)"), 1.0
            )

            # tp = tpT^T  [LP, C] via PE transposes (bf16 psum)
            ps_tp = ps.tile([P, C], BF16, tag="phT")
            for jt in range(CT):
                nc.tensor.transpose(
                    ps_tp[0:LP, jt * P : (jt + 1) * P],
                    tpT_sb[:, jt, 0:LP],
                    ident[:],
                )
            tp_sb = small.tile([P, C], BF16, tag="tp")
            nc.scalar.copy(tp_sb[0:LP, :], ps_tp[0:LP, :])

            # logits^T per j-tile (half-width psums) + fused exp -> E^T (bf16)
            et_sb = et_pool.tile([P, CT, C], BF16, tag="et")
            for jt in range(CT):
                lhsT = tp_sb[0:LP, jt * P : (jt + 1) * P]
                psl_a = ps.tile([P, HC], F32, tag="psl", bufs=2)
                nc.tensor.matmul(
                    psl_a[:], lhsT, gqt_sb[0:LP, 0:HC], start=True, stop=True
                )
                nc.scalar.activation(
                    et_sb[:, jt, 0:HC], psl_a[:], EXP, scale=SCALE
                )
                psl_b = ps.tile([P, HC], F32, tag="psl", bufs=2)
                nc.tensor.matmul(
                    psl_b[:], lhsT, gqt_sb[0:LP, HC:C], start=True, stop=True
                )
                nc.scalar.activation(
                    et_sb[:, jt, HC:C], psl_b[:], EXP, scale=SCALE
                )

            # EP = E @ [tp^T | 1]  [Ci, LP+1]; col LP = Z_i  (fp32 psum)
            ps_ep = ps.tile([P, CT, LP + 1], F32, tag="phF")
            for jt in range(CT):
                for it in range(CT):
                    nc.tensor.matmul(
                        ps_ep[:, it, :],
                        et_sb[:, jt, it * P : (it + 1) * P],
                        tpT_sb[:, jt, :],
                        start=(jt == 0),
                        stop=(jt == CT - 1),
                    )
            # EPs = EP / Z  (bf16)
            eps_sb = small.tile([P, CT, LP], BF16, tag="eps")
            rz = zp.tile([P, CT], F32, tag="rz")
            nc.vector.reciprocal(
                rz[:], ps_ep[:, :, LP : LP + 1].rearrange("p a b -> p (a b)")
            )
            for it in range(CT):
                nc.vector.tensor_scalar_mul(
                    eps_sb[:, it, :], ps_ep[:, it, 0:LP], rz[:, it : it + 1]
                )

            # EPsT = EPs^T  [LP, C] via PE transposes (bf16 psum)
            ps_epsT = ps.tile([P, C], BF16, tag="phT")
            for it in range(CT):
                nc.tensor.transpose(
                    ps_epsT[0:LP, it * P : (it + 1) * P],
                    eps_sb[:, it, :],
                    ident[:],
                )
            epsT_sb = small.tile([P, C], BF16, tag="epsT")
            nc.scalar.copy(epsT_sb[0:LP, :], ps_epsT[0:LP, :])

            # outT = qT + (gamma*G)^T @ EPsT  per s-tile halves; resid on DVE
            o_sb = outp.tile([P, ST, C], BF16, tag="o")
            for st in range(ST):
                lhsT = gg_sb[0:LP, st * P : (st + 1) * P]
                for h in range(2):
                    pso = ps.tile([P, HC], F32, tag="pso", bufs=2)
                    nc.tensor.matmul(
                        pso[:],
                        lhsT,
                        epsT_sb[0:LP, h * HC : (h + 1) * HC],
                        start=True,
                        stop=True,
                    )
                    nc.vector.tensor_tensor(
                        o_sb[:, st, h * HC : (h + 1) * HC],
                        pso[:],
                        qts[st][:, h * HC : (h + 1) * HC],
                        ADD,
                    )
            nc.sync.dma_start(out[b].rearrange("(st p) c -> p st c", p=P), o_sb[:])

    nc.compile()
    return nc


_NC = None


def _get_nc():
    global _NC
    if _NC is None:
        _NC = _build()
    return _NC


def _in_maps(img_feat, text_feat, W_txt, gamma):
    img = np.ascontiguousarray(
        img_feat.reshape(B, C, S), dtype=np.float32
    ).astype(BF)
    # txtt[p, k, b, l] = text[b, l, k*128+p]
    txtt = np.zeros((P, DT, B, LP), dtype=BF)
    t = np.asarray(text_feat, dtype=np.float32).astype(BF)  # [B, L, D]
    txtt[:, :, :, 0:L] = t.transpose(2, 0, 1).reshape(DT, P, B, L).transpose(1, 0, 2, 3)
    wt = np.ascontiguousarray(
        np.asarray(W_txt, dtype=np.float32).reshape(DT, P, C).transpose(1, 0, 2)
    ).astype(BF)
    g = _interp_matrix()
    gam = np.float32(np.asarray(gamma).reshape(-1)[0])
    gg = np.zeros((LP, S), dtype=BF)
    gg[0:L] = (gam * g).astype(BF)
    gt = np.zeros((P, ST, LP), dtype=BF)
    gt[:, :, 0:L] = g.T.reshape(ST, P, L).transpose(1, 0, 2).astype(BF)
    maps = []
    for m in range(N_CORES):
        sl = slice(m * B_CORE, (m + 1) * B_CORE)
        maps.append(
            {
                "img": np.ascontiguousarray(img[sl]),
                "txtt": np.ascontiguousarray(txtt[:, :, sl]),
                "wt": wt,
                "gg": gg,
                "gt": gt,
            }
        )
    return maps


def _run(in_maps, **kwargs):
    nc = _get_nc()
    return run_bass_kernel_spmd(nc, in_maps, core_ids=list(range(N_CORES)), **kwargs)


def kernel(img_feat, text_feat, W_txt, gamma):
    res = _run(_in_maps(img_feat, text_feat, W_txt, gamma))
    full = np.concatenate(
        [np.asarray(res.results[m]["out"]) for m in range(N_CORES)], axis=0
    )  # [B, S, C] bf16
    full = full.astype(np.float32).transpose(0, 2, 1)
    return np.ascontiguousarray(full.reshape(B, C, HH, WW), dtype=np.float32)
